# revision 25
# baseline (speedup 1.0000x reference)
"""GATv2 (3-layer, residual) Trainium2 kernel — 8-core SPMD, v4.

v4 vs v3:
 - 4 SWDGE queues; gather calls spread across queues (big win: random
   256B gathers go from ~30 GB/s to HBM-rate with 4 queues + big calls).
 - Quarter-major subtables: xl_full subtable t holds quarter t of every
   core's slots (8 cores x 3200 rows = 25600 <= int16). The AllGather is
   split into 4 quarter chunks so edge gathers of subtable t start as
   soon as chunk t lands, overlapping the collective with compute.
 - Global t-major chunk numbering (subtable-major, then dst tile) makes
   per-(group, subtable) index ranges contiguous for any grouping, so
   one idxj array serves per-layer group budgets.
 - idxj preloaded to SBUF once for the whole kernel; idxi provided in
   group-concatenated order per budget (one DMA per group).
 - xi gathered quarter-relative so it only waits on its own quarter's
   xl_own DMA, not the whole phase A.
 - Per-range (per-subtable) slab ops; per-tile scatter with po+pden
   packed into one PSUM bank.

Layout as v3: nodes dealt round-robin (node n -> core n%8, slot n//8);
features (c,h)-major on device.
"""

import sys

sys.path.insert(0, "/opt/trn_rl_repo")

import numpy as np
import ml_dtypes

import concourse.bacc as bacc
import concourse.bass as bass
import concourse.tile as tile
from concourse import mybir
from concourse import bass_utils
from concourse.masks import make_identity

BF16 = mybir.dt.bfloat16
F32 = mybir.dt.float32
I16 = mybir.dt.int16
AL = mybir.AluOpType
AF = mybir.ActivationFunctionType

NCORES = 8
P = 128
HEADS = 4
NEG_SLOPE = 0.2
RES_ALPHA = 0.1
EPS = 1e-16

QT = 25                  # tiles per quarter
NB = 4 * QT              # 100 node tiles per core
S = NB * P               # 12800 padded slots per core
QR = QT * P              # 3200 rows per quarter per core
SUB = NCORES * QR        # 25600 rows per subtable (int16-safe)
NSUB = 4

B01 = 48                 # group budget (chunks), layers 0/1 (F=128)
B2 = 24                  # group budget, layer 2 (F=256)
TCAP = 5                 # max tiles per group

nbf = ml_dtypes.bfloat16


# --------------------------------------------------------------------------
# Host-side preprocessing
# --------------------------------------------------------------------------

def _prep(edge_index, N):
    src = np.asarray(edge_index[0], dtype=np.int64)
    dst = np.asarray(edge_index[1], dtype=np.int64)
    E = src.shape[0]

    core_of = dst % NCORES
    slot_of = dst // NCORES              # < 12500 < S
    tile_of = slot_of // P
    dstl_of = slot_of % P

    csrc = src % NCORES
    ssrc = src // NCORES
    q_of = ssrc // QR                    # src quarter 0..3
    srow = csrc * QR + (ssrc % QR)       # row within subtable q_of

    # xi row: dst slot relative to its own quarter
    dq_of = slot_of // QR
    irow = slot_of - dq_of * QR + 0      # relative row; gather table is the
    # dst quarter of xl_own, so idx < QR*... actually table = quarter rows
    # [QR] per core: idx = slot - q'*QR < 3200

    # cell = (core, q, tile); counts
    cell = (core_of * NSUB + q_of) * NB + tile_of
    ncell = NCORES * NSUB * NB
    counts = np.bincount(cell, minlength=ncell).reshape(NCORES, NSUB, NB)
    cnt_chunks = np.ceil(counts / P).astype(np.int64).max(axis=0)  # [NSUB,NB]

    # global chunk numbering: t-major, then tile
    base = np.zeros((NSUB, NB), dtype=np.int64)
    nch = 0
    for t in range(NSUB):
        for k in range(NB):
            base[t, k] = nch
            nch += int(cnt_chunks[t, k])

    # per-edge chunk/lane (sorted by srow within each (core, t, k) cell)
    eorder = np.argsort(cell * (1 << 17) + srow, kind="stable")
    cnts = np.bincount(cell, minlength=ncell)
    offs = np.concatenate([[0], np.cumsum(cnts)])
    pos_in_cell = np.arange(E, dtype=np.int64) - offs[cell[eorder]]
    e_core = core_of[eorder]
    e_t = q_of[eorder]
    e_k = tile_of[eorder]
    e_chunk = base[e_t, e_k] + pos_in_cell // P
    e_pos = pos_in_cell % P

    idxj = np.zeros((NCORES, nch, P), dtype=np.int16)
    idxi = np.zeros((NCORES, nch, P), dtype=np.int16)
    dstl = np.full((NCORES, P, nch), -1.0, dtype=np.float32)
    idxj[e_core, e_chunk, e_pos] = srow[eorder].astype(np.int16)
    idxi[e_core, e_chunk, e_pos] = irow[eorder].astype(np.int16)
    dstl[e_core, e_pos, e_chunk] = dstl_of[eorder].astype(np.float32)

    def make_groups(budget):
        per_tile = cnt_chunks.sum(axis=0)       # [NB]
        groups = []
        ioff = 0
        for q in range(4):
            cur, tot = [], 0

            def flush():
                nonlocal cur, tot, ioff
                if not cur:
                    return
                ka, kb = cur[0], cur[-1]
                ranges, gbloffs, G = [], [], 0
                for t in range(NSUB):
                    rs = int(base[t, ka])
                    rl = int(cnt_chunks[t, ka:kb + 1].sum())
                    ranges.append((t, rs, rl))
                    gbloffs.append(G)
                    G += rl
                tiles = []
                for k in cur:
                    chl = []
                    for t in range(NSUB):
                        for j in range(int(cnt_chunks[t, k])):
                            gc = int(base[t, k]) + j
                            sc = gbloffs[t] + (gc - ranges[t][1])
                            chl.append((gc, sc))
                    if chl:
                        tiles.append((k, chl))
                groups.append(dict(q=q, ranges=ranges, offs=gbloffs, G=G,
                                   tiles=tiles, ioff=ioff))
                ioff += G
                cur, tot = [], 0

            for k in range(q * QT, (q + 1) * QT):
                c = int(per_tile[k])
                if cur and (tot + c > budget or len(cur) >= TCAP):
                    flush()
                cur.append(k)
                tot += c
            flush()
        return groups

    g01 = make_groups(B01)
    g2 = make_groups(B2)

    def order_of(groups):
        o = []
        for gi in groups:
            for (t, rs, rl) in gi["ranges"]:
                o.extend(range(rs, rs + rl))
        return np.asarray(o, dtype=np.int64)

    return dict(nch=nch, idxj=idxj, dstl=dstl,
                idxi01=idxi[:, order_of(g01), :],
                idxi2=idxi[:, order_of(g2), :],
                groups01=g01, groups2=g2)


def _wrap_idx_cols(idx_core):
    """[nch, 128] -> [16, nch*8] (the gather's wrapped idx layout)."""
    nch = idx_core.shape[0]
    a = idx_core.reshape(nch, 8, 16)
    return np.ascontiguousarray(
        np.transpose(a, (2, 0, 1)).reshape(16, nch * 8))


def _chperm(F):
    """original feature index (h-major) for each device feature (c-major)."""
    C = F // HEADS
    f = np.arange(F)
    c, h = f // HEADS, f % HEADS
    return h * C + c


# --------------------------------------------------------------------------
# Device program
# --------------------------------------------------------------------------

def build_program(nch, groups01, groups2, layers=3, repeat=1, sim=False, dbg=False, dbg_l=0, nogather=False, nocompute=False):
    KIN = 2                       # IN_C = 256
    HC = 128
    F2 = 256
    layer_F = [HC, HC, F2]
    layer_groups = [groups01, groups01, groups2]
    layer_B = [B01, B01, B2]
    BSLAB = B01 * HC              # == B2 * F2 slab elems

    nc = bacc.Bacc("TRN2", target_bir_lowering=False, debug=False,
                   num_devices=NCORES, num_swdge_queues=4)

    xt_d = [nc.dram_tensor(f"xt{i}", [P, S], BF16, kind="ExternalInput").ap()
            for i in range(KIN)]
    w0_d = [nc.dram_tensor(f"w0_{i}", [P, HC], BF16, kind="ExternalInput").ap()
            for i in range(KIN)]
    w1_d = nc.dram_tensor("w1", [P, HC], BF16, kind="ExternalInput").ap()
    w2_d = nc.dram_tensor("w2", [P, F2], BF16, kind="ExternalInput").ap()
    att_d = [nc.dram_tensor(f"att{i}", [P, layer_F[i]], BF16,
                            kind="ExternalInput").ap() for i in range(3)]
    b01_d = [nc.dram_tensor(f"b{i}", [P, 1], F32, kind="ExternalInput").ap()
             for i in range(2)]
    b2_d = nc.dram_tensor("b2", [64, 1], F32, kind="ExternalInput").ap()
    iota_d = nc.dram_tensor("iota", [P, P], BF16, kind="ExternalInput").ap()
    rmap_d = nc.dram_tensor("rmap", [HEADS, P], F32, kind="ExternalInput").ap()
    msum_d = [nc.dram_tensor(f"msum{i}", [P, 64], BF16,
                             kind="ExternalInput").ap() for i in range(2)]
    idxj_d = nc.dram_tensor("idxj", [16, nch * 8], I16,
                            kind="ExternalInput").ap()
    idxi01_d = nc.dram_tensor("idxi01", [16, nch * 8], I16,
                              kind="ExternalInput").ap()
    idxi2_d = nc.dram_tensor("idxi2", [16, nch * 8], I16,
                             kind="ExternalInput").ap()
    dstl_d = nc.dram_tensor("dstl", [P, nch], F32, kind="ExternalInput").ap()
    out_d = nc.dram_tensor("out", [S, 64], F32, kind="ExternalOutput").ap()
    if dbg:
        dbgxl_d = nc.dram_tensor("dbgxl", [S, HC], BF16,
                                 kind="ExternalOutput").ap()
        dbgxf_d = nc.dram_tensor("dbgxf", [SUB, HC], BF16,
                                 kind="ExternalOutput").ap()
        dbgh_d = nc.dram_tensor("dbgh", [P, S], BF16,
                                kind="ExternalOutput").ap()
        dbgxj_d = nc.dram_tensor("dbgxj", [P, 8192], BF16,
                                 kind="ExternalOutput").ap()
        dbgxi_d = nc.dram_tensor("dbgxi", [P, 8192], BF16,
                                 kind="ExternalOutput").ap()
        dbgex_d = nc.dram_tensor("dbgex", [P, 256], BF16,
                                 kind="ExternalOutput").ap()
        dbgpo_d = nc.dram_tensor("dbgpo", [P, P], F32,
                                 kind="ExternalOutput").ap()
        dbgpd_d = nc.dram_tensor("dbgpd", [HEADS, P], F32,
                                 kind="ExternalOutput").ap()

    import contextlib
    with tile.TileContext(nc) as tc, contextlib.ExitStack() as st:
        ec = st.enter_context
        cp = ec(tc.tile_pool(name="const", bufs=1))
        pp = ec(tc.tile_pool(name="pers", bufs=1))
        qb = ec(tc.tile_pool(name="qb", bufs=1))
        wp = ec(tc.tile_pool(name="stageA", bufs=3))
        gip = ec(tc.tile_pool(name="gidx", bufs=5))
        xjp = ec(tc.tile_pool(name="slabj", bufs=3))
        xip = ec(tc.tile_pool(name="slabi", bufs=3))
        axp = ec(tc.tile_pool(name="alex", bufs=2))
        zp = ec(tc.tile_pool(name="pm", bufs=6))
        tp = ec(tc.tile_pool(name="tails", bufs=2))
        psA = ec(tc.tile_pool(name="psA", bufs=1, space="PSUM"))
        psT = ec(tc.tile_pool(name="psT", bufs=1, space="PSUM"))
        psO = ec(tc.tile_pool(name="psO", bufs=2, space="PSUM"))
        psD = ec(tc.tile_pool(name="psD", bufs=1, space="PSUM"))
        psM = ec(tc.tile_pool(name="psM", bufs=1, space="PSUM"))
        dp = ec(tc.tile_pool(name="dram", bufs=1, space="DRAM"))
        if True:

            # ---- constants ----
            w0_sb = [cp.tile([P, HC], BF16, tag=f"w0_{i}", name=f"w0s{i}")
                     for i in range(KIN)]
            for i in range(KIN):
                nc.sync.dma_start(out=w0_sb[i][:], in_=w0_d[i][:])
            w1_sb = cp.tile([P, HC], BF16, tag="w1")
            nc.sync.dma_start(out=w1_sb[:], in_=w1_d[:])
            w2_sb = cp.tile([P, F2], BF16, tag="w2")
            nc.sync.dma_start(out=w2_sb[:], in_=w2_d[:])
            att_sb = []
            for i in range(3):
                t = cp.tile([P, layer_F[i]], BF16, tag=f"att{i}",
                            name=f"atts{i}")
                nc.sync.dma_start(out=t[:], in_=att_d[i][:])
                att_sb.append(t)
            b01_sb = []
            for i in range(2):
                t = cp.tile([P, 1], F32, tag=f"b{i}", name=f"bs{i}")
                nc.sync.dma_start(out=t[:], in_=b01_d[i][:])
                b01_sb.append(t)
            b2_sb = cp.tile([64, 1], F32, tag="b2")
            nc.sync.dma_start(out=b2_sb[:], in_=b2_d[:])
            iota_sb = cp.tile([P, P], BF16, tag="iota")
            nc.sync.dma_start(out=iota_sb[:], in_=iota_d[:])
            rmap_sb = cp.tile([HEADS, P], F32, tag="rmap")
            nc.sync.dma_start(out=rmap_sb[:], in_=rmap_d[:])
            msum_sb = []
            for i in range(2):
                t = cp.tile([P, 64], BF16, tag=f"msum{i}", name=f"msums{i}")
                nc.sync.dma_start(out=t[:], in_=msum_d[i][:])
                msum_sb.append(t)
            dstl_sb = cp.tile([P, nch], F32, tag="dstl")
            nc.sync.dma_start(out=dstl_sb[:], in_=dstl_d[:])
            ident = cp.tile([P, P], BF16, tag="ident")
            make_identity(nc, ident[:])
            identf = cp.tile([P, P], F32, tag="identf")
            make_identity(nc, identf[:])

            # ---- persistent ----
            x0T_sb = pp.tile([P, S], BF16, tag="x0T")
            hT_sb = pp.tile([P, S], BF16, tag="hT")
            nc.vector.memset(x0T_sb[:], 0.0)
            nc.vector.memset(hT_sb[:], 0.0)
            idxj_sb = pp.tile([P, nch * 8], I16, tag="idxj")
            for b in range(8):
                nc.sync.dma_start(out=idxj_sb[16 * b:16 * (b + 1), :],
                                  in_=idxj_d[:, :])

            xl_own = [dp.tile([S, layer_F[l]], BF16, tag=f"xlo{l}",
                              name=f"xlo{l}") for l in range(3)]
            xl_full = [[dp.tile([SUB, layer_F[l]], BF16, tag=f"xlf{l}_{q}",
                                name=f"xlf{l}_{q}")
                        for q in range(4)] for l in range(3)]
            idxi_rep = [dp.tile([P, nch * 8], I16, tag=f"iir{i}",
                                name=f"iir{i}") for i in range(2)]
            for i, d in enumerate([idxi01_d, idxi2_d]):
                for b in range(8):
                    nc.sync.dma_start(out=idxi_rep[i][16 * b:16 * (b + 1), :],
                                      in_=d[:, :])

            for l in [ll for _ in range(repeat) for ll in range(layers)]:
                F = layer_F[l]
                C = F // HEADS
                NFS = F // P
                groups = layer_groups[l]
                B = layer_B[l]
                irep = idxi_rep[0] if l < 2 else idxi_rep[1]

                # ---------- phase A + chunked AllGather ----------
                for q in range(4):
                    qbuf = qb.tile([P, QT * F2], BF16, tag="qbuf")
                    for kk in range(QT):
                        k = q * QT + kk
                        ps = psA.tile([P, F2], F32, tag="psA")
                        if l == 0:
                            for i in range(KIN):
                                xa = wp.tile([P, P], BF16, tag=f"xta{i}",
                                             name=f"xta{i}")
                                nc.sync.dma_start(
                                    out=xa[:],
                                    in_=xt_d[i][:, k * P:(k + 1) * P])
                                nc.tensor.matmul(ps[:, :F], lhsT=xa[:],
                                                 rhs=w0_sb[i][:],
                                                 start=(i == 0),
                                                 stop=(i == KIN - 1))
                        else:
                            w_sb = w1_sb if l == 1 else w2_sb
                            nc.tensor.matmul(ps[:, :F],
                                             lhsT=hT_sb[:, k * P:(k + 1) * P],
                                             rhs=w_sb[:], start=True,
                                             stop=True)
                        stage = qbuf[:, kk * F:(kk + 1) * F]
                        nc.scalar.copy(stage, ps[:, :F])
                        if l == 0:
                            ptx = psT.tile([P, P], BF16, tag="ptx",
                                           name="ptx")
                            nc.tensor.transpose(ptx[:], stage, ident[:])
                            nc.vector.tensor_scalar(
                                out=x0T_sb[:, k * P:(k + 1) * P], in0=ptx[:],
                                scalar1=RES_ALPHA, scalar2=None, op0=AL.mult)
                    # quarter -> xl_own rows [q*QR, (q+1)*QR)
                    nc.sync.dma_start(
                        out=xl_own[l][q * QR:(q + 1) * QR, :]
                        .rearrange("(k p) f -> p k f", p=P),
                        in_=qbuf[:, :QT * F]
                        .rearrange("p (k f) -> p k f", f=F))
                    if sim:
                        for c in range(NCORES):
                            nc.sync.dma_start(
                                out=xl_full[l][q][c * QR:(c + 1) * QR, :],
                                in_=xl_own[l][q * QR:(q + 1) * QR, :])
                    else:
                        nc.gpsimd.collective_compute(
                            "AllGather", AL.bypass,
                            replica_groups=[list(range(NCORES))],
                            ins=[xl_own[l][q * QR:(q + 1) * QR, :].opt()],
                            outs=[xl_full[l][q].opt()],
                        )

                # ---------- edge phase ----------
                for gidx, gi in enumerate(groups):
                    G = gi["G"]
                    gq = gi["q"]
                    ii = gip.tile([P, B01 * 8], I16, tag="ii")
                    nc.sync.dma_start(
                        out=ii[:, :G * 8],
                        in_=irep[:, gi["ioff"] * 8:(gi["ioff"] + G) * 8])
                    xj = xjp.tile([P, BSLAB], BF16, tag="xj")
                    xi = xip.tile([P, BSLAB], BF16, tag="xi")
                    if not nogather:
                      nc.gpsimd.dma_gather(
                        out_ap=xi[:, :G * F].rearrange("p (c f) -> p c f",
                                                       f=F),
                        in_ap=xl_own[l][gq * QR:(gq + 1) * QR, :],
                        idxs_ap=ii[:, :G * 8],
                        num_idxs=G * P, num_idxs_reg=G * P,
                        elem_size=F, single_packet=False,
                        queue_num=(gidx + 2) % 4)
                    for (t, rs, rl) in gi["ranges"]:
                        if rl == 0:
                            continue
                        sc0 = gi["offs"][t]
                        if nogather:
                            continue
                        nc.gpsimd.dma_gather(
                            out_ap=xj[:, sc0 * F:(sc0 + rl) * F]
                            .rearrange("p (c f) -> p c f", f=F),
                            in_ap=xl_full[l][t][:, :],
                            idxs_ap=idxj_sb[:, rs * 8:(rs + rl) * 8],
                            num_idxs=rl * P, num_idxs_reg=rl * P,
                            elem_size=F, single_packet=False,
                            queue_num=(t + gidx) % 4)

                    if dbg and l == dbg_l and gidx == 2:
                        nc.sync.dma_start(out=dbgxj_d[:], in_=xj[:])
                        nc.sync.dma_start(out=dbgxi_d[:], in_=xi[:])
                    if nocompute:
                        continue
                    al = axp.tile([P, B01 * HEADS], BF16, tag="al")
                    ex = axp.tile([P, B01 * HEADS], BF16, tag="ex")
                    for (t, rs, rl) in gi["ranges"]:
                        if rl == 0:
                            continue
                        sc0 = gi["offs"][t]
                        ea = xi[:, sc0 * F:(sc0 + rl) * F]
                        xjr = xj[:, sc0 * F:(sc0 + rl) * F]
                        nc.vector.tensor_tensor(out=ea, in0=ea, in1=xjr,
                                                op=AL.add)
                        nc.vector.scalar_tensor_tensor(
                            out=ea, in0=ea, scalar=NEG_SLOPE, in1=ea,
                            op0=AL.mult, op1=AL.max)
                        nc.vector.tensor_tensor(
                            out=ea.rearrange("p (g f) -> p g f", f=F),
                            in0=ea.rearrange("p (g f) -> p g f", f=F),
                            in1=att_sb[l][:].unsqueeze(1)
                                .broadcast_to([P, rl, F]),
                            op=AL.mult)
                        ea4 = ea.rearrange("p (g c h) -> p g c h",
                                           h=HEADS, c=C)
                        w = C // 2
                        while w > 1:
                            nc.vector.tensor_tensor(
                                out=ea4[:, :, 0:w, :], in0=ea4[:, :, 0:w, :],
                                in1=ea4[:, :, w:2 * w, :], op=AL.add)
                            w //= 2
                        alr = al[:, sc0 * HEADS:(sc0 + rl) * HEADS]
                        nc.vector.tensor_tensor(
                            out=alr.rearrange("p (g h) -> p g h", h=HEADS),
                            in0=ea4[:, :, 0, :], in1=ea4[:, :, 1, :],
                            op=AL.add)
                        exr = ex[:, sc0 * HEADS:(sc0 + rl) * HEADS]
                        nc.scalar.activation(exr, alr, AF.Exp)
                        nc.vector.tensor_tensor(
                            out=xjr.rearrange("p (g c h) -> p g c h",
                                              h=HEADS, c=C),
                            in0=xjr.rearrange("p (g c h) -> p g c h",
                                              h=HEADS, c=C),
                            in1=exr.rearrange("p (g h) -> p g h", h=HEADS)
                                .unsqueeze(2).broadcast_to([P, rl, C, HEADS]),
                            op=AL.mult)

                    if dbg and l == dbg_l and gidx == 2:
                        nc.sync.dma_start(out=dbgex_d[:], in_=ex[:])
                    # ---------- scatter + tails per tile ----------
                    for (k, chl) in gi["tiles"]:
                        po = [psO.tile([P, P], F32, tag=f"po{fs}",
                                       name=f"po{fs}")
                              for fs in range(NFS)]
                        pden = psD.tile([HEADS, P], F32, tag="pden")
                        for ci, (gc, sc) in enumerate(chl):
                            first = ci == 0
                            last = ci == len(chl) - 1
                            Pm = zp.tile([P, P], BF16, tag="Pm")
                            nc.vector.tensor_scalar(
                                out=Pm[:], in0=iota_sb[:],
                                scalar1=dstl_sb[:, gc:gc + 1], scalar2=None,
                                op0=AL.is_equal)
                            for fs in range(NFS):
                                nc.tensor.matmul(
                                    po[fs][:],
                                    lhsT=xj[:, sc * F + fs * P:
                                            sc * F + (fs + 1) * P],
                                    rhs=Pm[:], start=first, stop=last)
                            nc.tensor.matmul(
                                pden[:],
                                lhsT=ex[:, sc * HEADS:(sc + 1) * HEADS],
                                rhs=Pm[:], start=first, stop=last)

                        if dbg and l == dbg_l and gidx == 2 and k == gi["tiles"][0][0]:
                            pocp = tp.tile([P, P], F32, tag="pocp")
                            nc.scalar.copy(pocp[:], po[0][:])
                            nc.sync.dma_start(out=dbgpo_d[:], in_=pocp[:])
                            pdcp = tp.tile([HEADS, P], F32, tag="pdcp")
                            nc.scalar.copy(pdcp[:], pden[:])
                            nc.sync.dma_start(out=dbgpd_d[:], in_=pdcp[:])
                        # ---- tile tail ----
                        rec = tp.tile([HEADS, P], F32, tag="rec")
                        nc.vector.tensor_scalar(out=rec[:], in0=pden[:],
                                                scalar1=EPS, scalar2=None,
                                                op0=AL.add)
                        nc.vector.reciprocal(rec[:], rec[:])
                        scale = (1.0 - RES_ALPHA) if l < 2 else (1.0 / HEADS)
                        nc.vector.tensor_scalar(out=rec[:], in0=rec[:],
                                                scalar1=scale, scalar2=None,
                                                op0=AL.mult)
                        cols = slice(k * P, (k + 1) * P)
                        if l < 2:
                            prep = psM.tile([P, P], F32, tag="psM",
                                            name="prep")
                            nc.tensor.matmul(prep[:], lhsT=rmap_sb[:],
                                             rhs=rec[:], start=True,
                                             stop=True)
                            rep = tp.tile([P, P], F32, tag="rep")
                            nc.scalar.copy(rep[:], prep[:])
                            u = tp.tile([P, P], F32, tag="u")
                            nc.vector.tensor_tensor(out=u[:], in0=po[0][:],
                                                    in1=rep[:], op=AL.mult)
                            nc.scalar.activation(u[:], u[:], AF.Identity,
                                                 bias=b01_sb[l][:, 0:1])
                            nc.vector.tensor_tensor(out=u[:], in0=u[:],
                                                    in1=x0T_sb[:, cols],
                                                    op=AL.add)
                            mn = tp.tile([P, P], F32, tag="mn")
                            nc.vector.tensor_scalar(out=mn[:], in0=u[:],
                                                    scalar1=0.0,
                                                    scalar2=None, op0=AL.min)
                            nc.scalar.activation(mn[:], mn[:], AF.Exp)
                            hh = tp.tile([P, P], F32, tag="hh")
                            nc.vector.scalar_tensor_tensor(
                                out=hh[:], in0=u[:], scalar=0.0,
                                in1=mn[:], op0=AL.max, op1=AL.add)
                            nc.vector.tensor_scalar(
                                out=hT_sb[:, cols], in0=hh[:],
                                scalar1=-1.0, scalar2=None, op0=AL.add)
                        else:
                            tsb = []
                            for fs in range(2):
                                prep = psM.tile([P, P], F32, tag="psM",
                                                name="prep")
                                nc.tensor.matmul(prep[:], lhsT=rmap_sb[:],
                                                 rhs=rec[:], start=True,
                                                 stop=True)
                                rep = tp.tile([P, P], F32, tag="rep")
                                nc.scalar.copy(rep[:], prep[:])
                                tt = tp.tile([P, P], BF16, tag=f"t{fs}",
                                             name=f"tsb{fs}")
                                nc.vector.tensor_tensor(
                                    out=tt[:], in0=po[fs][:],
                                    in1=rep[:], op=AL.mult)
                                tsb.append(tt)
                            pmo = psM.tile([P, P], F32, tag="psM",
                                           name="pmo")
                            nc.tensor.matmul(pmo[0:64, :],
                                             lhsT=msum_sb[0][:],
                                             rhs=tsb[0][:], start=True,
                                             stop=False)
                            nc.tensor.matmul(pmo[0:64, :],
                                             lhsT=msum_sb[1][:],
                                             rhs=tsb[1][:], start=False,
                                             stop=True)
                            ob = tp.tile([64, P], F32, tag="ob")
                            nc.scalar.activation(ob[:], pmo[0:64, :],
                                                 AF.Identity,
                                                 bias=b2_sb[:, 0:1])
                            pot = psM.tile([P, P], F32, tag="psM",
                                           name="pot")
                            nc.tensor.transpose(pot[:, 0:64], ob[:],
                                                identf[:64, :64])
                            orow = tp.tile([P, 64], F32, tag="orow")
                            nc.scalar.copy(orow[:], pot[:, 0:64])
                            nc.sync.dma_start(
                                out=out_d[k * P:(k + 1) * P, :],
                                in_=orow[:])

            if dbg:
                nc.sync.dma_start(out=dbgxl_d[:], in_=xl_own[0][:, :])
                nc.sync.dma_start(out=dbgxf_d[:], in_=xl_full[0][0][:, :])
                nc.sync.dma_start(out=dbgh_d[:], in_=hT_sb[:])

    nc.compile()
    return nc


# --------------------------------------------------------------------------
# kernel() entry
# --------------------------------------------------------------------------

def prepare(x, edge_index, W0, b0, att0, W1, b1, att1, W2, b2, att2,
            _layers=3, _repeat=1, _sim=False, _dbg=False, _dbg_l=0, _nogather=False, _nocompute=False):
    x = np.asarray(x, dtype=np.float32)
    N, IN_C = x.shape
    pr = _prep(edge_index, N)
    nch = pr["nch"]

    nc = build_program(nch, pr["groups01"], pr["groups2"],
                       layers=_layers, repeat=_repeat, sim=_sim, dbg=_dbg, dbg_l=_dbg_l, nogather=_nogather, nocompute=_nocompute)

    KIN = IN_C // P
    p128 = _chperm(128)     # device feature -> original feature (F=128)
    p256 = _chperm(256)

    W0p = np.asarray(W0, np.float32)[:, p128]           # cols -> (c,h)
    W1p = np.asarray(W1, np.float32)[p128][:, p128]     # rows from hT, cols
    W2p = np.asarray(W2, np.float32)[p128][:, p256]

    common = {}
    for i in range(KIN):
        common[f"w0_{i}"] = W0p[i * P:(i + 1) * P, :].astype(nbf)
    common["w1"] = W1p.astype(nbf)
    common["w2"] = W2p.astype(nbf)

    def rep_att(att, F):
        flat = np.asarray(att, np.float32).reshape(-1)[_chperm(F)]
        return np.repeat(flat[None, :], P, axis=0).astype(nbf)

    common["att0"] = rep_att(att0, 128)
    common["att1"] = rep_att(att1, 128)
    common["att2"] = rep_att(att2, 256)
    common["b0"] = ((1.0 - RES_ALPHA) * np.asarray(b0, np.float32)[p128]
                    ).reshape(P, 1)
    common["b1"] = ((1.0 - RES_ALPHA) * np.asarray(b1, np.float32)[p128]
                    ).reshape(P, 1)
    common["b2"] = np.asarray(b2, np.float32).reshape(64, 1)
    common["iota"] = np.tile(np.arange(P, dtype=np.float32)[None, :],
                             (P, 1)).astype(nbf)
    # head of device feature f is f % HEADS (same map for all layers)
    r0 = np.zeros((HEADS, P), np.float32)
    for f in range(P):
        r0[f % HEADS, f] = 1.0
    common["rmap"] = r0
    # l2 head-mean: device feature g=fs*128+f maps to out channel g//HEADS
    for fs in range(2):
        m = np.zeros((P, 64), np.float32)
        for f in range(P):
            m[f, (fs * P + f) // HEADS] = 1.0
        common[f"msum{fs}"] = m.astype(nbf)

    in_maps = []
    for c in range(NCORES):
        m = dict(common)
        nodes = np.arange(c, N, NCORES, dtype=np.int64)
        xc = np.zeros((S, IN_C), dtype=np.float32)
        xc[:len(nodes)] = x[nodes]
        xct = np.ascontiguousarray(xc.T).astype(nbf)
        for i in range(KIN):
            m[f"xt{i}"] = xct[i * P:(i + 1) * P, :].copy()
        m["idxj"] = _wrap_idx_cols(pr["idxj"][c])
        m["idxi01"] = _wrap_idx_cols(pr["idxi01"][c])
        m["idxi2"] = _wrap_idx_cols(pr["idxi2"][c])
        m["dstl"] = pr["dstl"][c]
        in_maps.append(m)

    def assemble(per_core_out):
        out = np.zeros((N, 64), dtype=np.float32)
        for c in range(NCORES):
            nodes = np.arange(c, N, NCORES, dtype=np.int64)
            out[nodes] = per_core_out[c][:len(nodes)]
        return out

    return nc, in_maps, assemble


def kernel(x, edge_index, W0, b0, att0, W1, b1, att1, W2, b2, att2, **kw):
    nc, in_maps, assemble = prepare(x, edge_index, W0, b0, att0, W1, b1,
                                    att1, W2, b2, att2, **kw)
    # run twice: the very first execution on freshly-initialized devices has
    # been observed (rarely) to race; the second run is authoritative.
    bass_utils.run_bass_kernel_spmd(nc, in_maps,
                                    core_ids=list(range(NCORES)))
    res = bass_utils.run_bass_kernel_spmd(nc, in_maps,
                                          core_ids=list(range(NCORES)))
    return assemble([res.results[c]["out"] for c in range(NCORES)])
